# revision 1
# baseline (speedup 1.0000x reference)
"""Fused TP-allreduce + bias/residual add + RMSNorm for Trainium2 (8 NeuronCores).

Strategy: the reference computes sum(x, axis=0) over the tp axis, then a
fused epilogue (bias + residual add, RMSNorm) on the [tokens, hidden] result.
Since this kernel receives the FULL inputs and distributes them itself, we
shard by TOKENS instead of tp-rank: core i gets x[:, i*1024:(i+1)*1024, :]
(all 8 tp slices for its token range) plus the matching residual rows and the
replicated bias/norm_weight. Each core reduces its 8 local slices and runs
the epilogue on its token shard — no inter-core communication at all. The
host concatenates the per-core output shards. This turns the problem into a
pure memory-bound streaming kernel (~176 MB HBM traffic per core).
"""

import numpy as np

TP = 8
TOKENS = 8192
HIDDEN = 4096
N_CORES = 8
TOK_PER_CORE = TOKENS // N_CORES  # 1024
P = 128  # SBUF partitions (token-tile height)
N_TILES = TOK_PER_CORE // P  # 8
EPS = 1e-6

_COMPILED = {}


def _broadcast_ap(ap, parts):
    """View a [N] DRAM AP as [parts, N] with partition stride 0."""
    import concourse.bass as bass

    return bass.AP(tensor=ap.tensor, offset=ap.offset, ap=[[0, parts]] + list(ap.ap))


def _build():
    import concourse.bacc as bacc
    import concourse.tile as tile
    from concourse import mybir

    f32 = mybir.dt.float32
    nc = bacc.Bacc(
        "TRN2",
        target_bir_lowering=False,
        debug=False,
        enable_asserts=False,
        num_devices=N_CORES,
    )

    x = nc.dram_tensor("x", [TP, TOK_PER_CORE, HIDDEN], f32, kind="ExternalInput").ap()
    residual = nc.dram_tensor(
        "residual", [TOK_PER_CORE, HIDDEN], f32, kind="ExternalInput"
    ).ap()
    bias = nc.dram_tensor("bias", [HIDDEN], f32, kind="ExternalInput").ap()
    weight = nc.dram_tensor("norm_weight", [HIDDEN], f32, kind="ExternalInput").ap()
    norm_out = nc.dram_tensor(
        "norm_out", [TOK_PER_CORE, HIDDEN], f32, kind="ExternalOutput"
    ).ap()
    residual_out = nc.dram_tensor(
        "residual_out", [TOK_PER_CORE, HIDDEN], f32, kind="ExternalOutput"
    ).ap()

    with tile.TileContext(nc) as tc:
        with (
            tc.tile_pool(name="consts", bufs=1) as consts,
            tc.tile_pool(name="xp", bufs=4) as xp,
            tc.tile_pool(name="accp", bufs=2) as accp,
            tc.tile_pool(name="resp", bufs=2) as resp,
            tc.tile_pool(name="noutp", bufs=2) as noutp,
            tc.tile_pool(name="statp", bufs=4) as statp,
        ):
            bias_t = consts.tile([P, HIDDEN], f32)
            nc.gpsimd.dma_start(out=bias_t[:], in_=_broadcast_ap(bias, P))
            w_t = consts.tile([P, HIDDEN], f32)
            nc.gpsimd.dma_start(out=w_t[:], in_=_broadcast_ap(weight, P))
            eps_t = consts.tile([P, 1], f32)
            nc.vector.memset(eps_t[:], EPS)

            n_groups = HIDDEN // nc.vector.BN_STATS_FMAX  # 8 subgroups of 512

            for it in range(N_TILES):
                t0 = it * P

                res_t = resp.tile([P, HIDDEN], f32)
                nc.sync.dma_start(out=res_t[:], in_=residual[t0 : t0 + P, :])

                x_tiles = []
                for i in range(TP):
                    xt = xp.tile([P, HIDDEN], f32)
                    nc.sync.dma_start(out=xt[:], in_=x[i, t0 : t0 + P, :])
                    x_tiles.append(xt)

                # Serial accumulate: acc = x0 + x1; acc += x2..x7
                acc = accp.tile([P, HIDDEN], f32)
                nc.vector.tensor_add(acc[:], x_tiles[0][:], x_tiles[1][:])
                for i in range(2, TP):
                    nc.vector.tensor_add(acc[:], acc[:], x_tiles[i][:])
                # + residual + bias (residual_out)
                nc.vector.tensor_add(acc[:], acc[:], res_t[:])
                nc.vector.tensor_add(acc[:], acc[:], bias_t[:])

                nc.sync.dma_start(out=residual_out[t0 : t0 + P, :], in_=acc[:])

                # mean(x^2) = var + mean^2 via bn_stats/bn_aggr
                stats = statp.tile([P, n_groups, nc.vector.BN_STATS_DIM], f32)
                for g in range(n_groups):
                    nc.vector.bn_stats(
                        out=stats[:, g, :], in_=acc[:, g * 512 : (g + 1) * 512]
                    )
                mv = statp.tile([P, nc.vector.BN_AGGR_DIM], f32)
                nc.vector.bn_aggr(out=mv[:], in_=stats[:])

                ms = statp.tile([P, 1], f32)
                nc.vector.tensor_mul(ms[:], mv[:, 0:1], mv[:, 0:1])
                nc.vector.tensor_add(ms[:], ms[:], mv[:, 1:2])
                # rstd = 1/sqrt(ms + eps)
                rstd = statp.tile([P, 1], f32)
                nc.scalar.activation(
                    out=rstd[:],
                    in_=ms[:],
                    func=mybir.ActivationFunctionType.Sqrt,
                    bias=eps_t[:],
                )
                nc.vector.reciprocal(out=rstd[:], in_=rstd[:])

                # norm_out = residual_out * rstd * norm_weight
                nout = noutp.tile([P, HIDDEN], f32)
                nc.vector.tensor_scalar_mul(nout[:], acc[:], rstd[:])
                nc.vector.tensor_mul(nout[:], nout[:], w_t[:])
                nc.sync.dma_start(out=norm_out[t0 : t0 + P, :], in_=nout[:])

    nc.compile()
    return nc


def _get_compiled():
    if "nc" not in _COMPILED:
        _COMPILED["nc"] = _build()
    return _COMPILED["nc"]


def _shard_inputs(x, bias, residual, norm_weight):
    x = np.ascontiguousarray(np.asarray(x, dtype=np.float32))
    bias = np.ascontiguousarray(np.asarray(bias, dtype=np.float32))
    residual = np.ascontiguousarray(np.asarray(residual, dtype=np.float32))
    norm_weight = np.ascontiguousarray(np.asarray(norm_weight, dtype=np.float32))
    in_maps = []
    for c in range(N_CORES):
        lo, hi = c * TOK_PER_CORE, (c + 1) * TOK_PER_CORE
        in_maps.append(
            {
                "x": np.ascontiguousarray(x[:, lo:hi, :]),
                "residual": residual[lo:hi],
                "bias": bias,
                "norm_weight": norm_weight,
            }
        )
    return in_maps


def run(inputs, trace=False):
    """Run the SPMD kernel. Returns ((norm_out, residual_out), BassKernelResults)."""
    from concourse.bass_utils import run_bass_kernel_spmd

    nc = _get_compiled()
    in_maps = _shard_inputs(
        inputs["x"], inputs["bias"], inputs["residual"], inputs["norm_weight"]
    )
    res = run_bass_kernel_spmd(nc, in_maps, core_ids=list(range(N_CORES)), trace=trace)
    norm = np.concatenate([res.results[c]["norm_out"] for c in range(N_CORES)], axis=0)
    rout = np.concatenate(
        [res.results[c]["residual_out"] for c in range(N_CORES)], axis=0
    )
    return (norm, rout), res


def kernel(x, bias, residual, norm_weight, **_unused):
    (norm, rout), _ = run(
        {"x": x, "bias": bias, "residual": residual, "norm_weight": norm_weight}
    )
    return norm, rout


# revision 3
# speedup vs baseline: 1.0953x; 1.0953x over previous
"""Fused TP-allreduce + bias/residual add + RMSNorm for Trainium2 (8 NeuronCores).

Strategy: the reference computes sum(x, axis=0) over the tp axis, then a
fused epilogue (bias + residual add, RMSNorm) on the [tokens, hidden] result.
Since this kernel receives the FULL inputs and distributes them itself, we
shard by TOKENS instead of tp-rank: core i gets x[:, i*1024:(i+1)*1024, :]
(all 8 tp slices for its token range) plus the matching residual rows and the
replicated bias/norm_weight. Each core reduces its 8 local slices and runs
the epilogue on its token shard — no inter-core communication at all. The
host concatenates the per-core output shards. This turns the problem into a
pure memory-bound streaming kernel (~176 MB HBM traffic per core).
"""

import numpy as np

TP = 8
TOKENS = 8192
HIDDEN = 4096
N_CORES = 8
TOK_PER_CORE = TOKENS // N_CORES  # 1024
P = 128  # SBUF partitions (token-tile height)
N_TILES = TOK_PER_CORE // P  # 8
EPS = 1e-6

_COMPILED = {}


def _broadcast_ap(ap, parts):
    """View a [N] DRAM AP as [parts, N] with partition stride 0."""
    import concourse.bass as bass

    return bass.AP(tensor=ap.tensor, offset=ap.offset, ap=[[0, parts]] + list(ap.ap))


def _build():
    import concourse.bacc as bacc
    import concourse.tile as tile
    from concourse import mybir

    f32 = mybir.dt.float32
    bf16 = mybir.dt.bfloat16
    nc = bacc.Bacc(
        "TRN2",
        target_bir_lowering=False,
        debug=False,
        enable_asserts=False,
        num_devices=N_CORES,
    )

    x = nc.dram_tensor("x", [TP, TOK_PER_CORE, HIDDEN], f32, kind="ExternalInput").ap()
    residual = nc.dram_tensor(
        "residual", [TOK_PER_CORE, HIDDEN], f32, kind="ExternalInput"
    ).ap()
    bias = nc.dram_tensor("bias", [HIDDEN], f32, kind="ExternalInput").ap()
    weight = nc.dram_tensor("norm_weight", [HIDDEN], f32, kind="ExternalInput").ap()
    norm_out = nc.dram_tensor(
        "norm_out", [TOK_PER_CORE, HIDDEN], f32, kind="ExternalOutput"
    ).ap()
    residual_out = nc.dram_tensor(
        "residual_out", [TOK_PER_CORE, HIDDEN], f32, kind="ExternalOutput"
    ).ap()

    with tile.TileContext(nc) as tc:
        with (
            tc.tile_pool(name="consts", bufs=1) as consts,
            tc.tile_pool(name="xp", bufs=6) as xp,
            tc.tile_pool(name="accp", bufs=2) as accp,
            tc.tile_pool(name="routp", bufs=2) as routp,
            tc.tile_pool(name="resp", bufs=2) as resp,
            tc.tile_pool(name="noutp", bufs=2) as noutp,
            tc.tile_pool(name="statp", bufs=4) as statp,
        ):
            bias_t = consts.tile([P, HIDDEN], f32)
            nc.gpsimd.dma_start(out=bias_t[:], in_=_broadcast_ap(bias, P))
            w_t = consts.tile([P, HIDDEN], f32)
            nc.gpsimd.dma_start(out=w_t[:], in_=_broadcast_ap(weight, P))
            eps_t = consts.tile([P, 1], f32)
            nc.vector.memset(eps_t[:], EPS)

            n_groups = HIDDEN // nc.vector.BN_STATS_FMAX  # 8 subgroups of 512

            for it in range(N_TILES):
                t0 = it * P

                res_t = resp.tile([P, HIDDEN], f32)
                nc.sync.dma_start(out=res_t[:], in_=residual[t0 : t0 + P, :])

                # Cast-DMA (SWDGE) x slices f32->bf16: the tp-sum adds then
                # run in the DVE 2x (16-bit) perf mode.
                x_tiles = []
                for i in range(TP):
                    xt = xp.tile([P, HIDDEN], bf16)
                    nc.gpsimd.dma_start(out=xt[:], in_=x[i, t0 : t0 + P, :])
                    x_tiles.append(xt)

                # Serial accumulate in bf16: s = x0 + x1; s += x2..x7
                s = accp.tile([P, HIDDEN], bf16)
                nc.vector.tensor_add(s[:], x_tiles[0][:], x_tiles[1][:])
                for i in range(2, TP):
                    nc.vector.tensor_add(s[:], s[:], x_tiles[i][:])
                # residual_out = sum + residual + bias (f32)
                rout = routp.tile([P, HIDDEN], f32)
                nc.vector.tensor_add(rout[:], s[:], res_t[:])
                nc.vector.tensor_add(rout[:], rout[:], bias_t[:])

                nc.sync.dma_start(out=residual_out[t0 : t0 + P, :], in_=rout[:])

                # mean(x^2) = var + mean^2 via bn_stats/bn_aggr
                stats = statp.tile([P, n_groups, nc.vector.BN_STATS_DIM], f32)
                for g in range(n_groups):
                    nc.vector.bn_stats(
                        out=stats[:, g, :], in_=rout[:, g * 512 : (g + 1) * 512]
                    )
                mv = statp.tile([P, nc.vector.BN_AGGR_DIM], f32)
                nc.vector.bn_aggr(out=mv[:], in_=stats[:])

                # ms = mean^2 + var in one tensor_scalar
                ms = statp.tile([P, 1], f32)
                nc.vector.tensor_scalar(
                    out=ms[:],
                    in0=mv[:, 0:1],
                    scalar1=mv[:, 0:1],
                    scalar2=mv[:, 1:2],
                    op0=mybir.AluOpType.mult,
                    op1=mybir.AluOpType.add,
                )
                # rstd = 1/sqrt(ms + eps)
                rstd = statp.tile([P, 1], f32)
                nc.scalar.activation(
                    out=rstd[:],
                    in_=ms[:],
                    func=mybir.ActivationFunctionType.Sqrt,
                    bias=eps_t[:],
                )
                nc.vector.reciprocal(out=rstd[:], in_=rstd[:])

                # norm_out = residual_out * rstd * norm_weight
                # (rstd scale on the Scalar engine; weight mul on DVE)
                nout = noutp.tile([P, HIDDEN], f32)
                nc.scalar.activation(
                    out=nout[:],
                    in_=rout[:],
                    func=mybir.ActivationFunctionType.Copy,
                    scale=rstd[:],
                )
                nc.vector.tensor_mul(nout[:], nout[:], w_t[:])
                nc.scalar.dma_start(out=norm_out[t0 : t0 + P, :], in_=nout[:])

    nc.compile()
    return nc


def _get_compiled():
    if "nc" not in _COMPILED:
        _COMPILED["nc"] = _build()
    return _COMPILED["nc"]


def _shard_inputs(x, bias, residual, norm_weight):
    x = np.ascontiguousarray(np.asarray(x, dtype=np.float32))
    bias = np.ascontiguousarray(np.asarray(bias, dtype=np.float32))
    residual = np.ascontiguousarray(np.asarray(residual, dtype=np.float32))
    norm_weight = np.ascontiguousarray(np.asarray(norm_weight, dtype=np.float32))
    in_maps = []
    for c in range(N_CORES):
        lo, hi = c * TOK_PER_CORE, (c + 1) * TOK_PER_CORE
        in_maps.append(
            {
                "x": np.ascontiguousarray(x[:, lo:hi, :]),
                "residual": residual[lo:hi],
                "bias": bias,
                "norm_weight": norm_weight,
            }
        )
    return in_maps


def run(inputs, trace=False):
    """Run the SPMD kernel. Returns ((norm_out, residual_out), BassKernelResults)."""
    from concourse.bass_utils import run_bass_kernel_spmd

    nc = _get_compiled()
    in_maps = _shard_inputs(
        inputs["x"], inputs["bias"], inputs["residual"], inputs["norm_weight"]
    )
    res = run_bass_kernel_spmd(nc, in_maps, core_ids=list(range(N_CORES)), trace=trace)
    norm = np.concatenate([res.results[c]["norm_out"] for c in range(N_CORES)], axis=0)
    rout = np.concatenate(
        [res.results[c]["residual_out"] for c in range(N_CORES)], axis=0
    )
    return (norm, rout), res


def kernel(x, bias, residual, norm_weight, **_unused):
    (norm, rout), _ = run(
        {"x": x, "bias": bias, "residual": residual, "norm_weight": norm_weight}
    )
    return norm, rout
